# revision 14
# baseline (speedup 1.0000x reference)
import os
import sys
import types

import numpy as np

sys.path.insert(0, "/opt/trn_rl_repo")

import ml_dtypes  # noqa: E402
import concourse.mybir as mybir  # noqa: E402
import concourse.tile as tile  # noqa: E402
from concourse import bacc  # noqa: E402
from concourse.bass import ds, ts  # noqa: E402
from concourse.bass_utils import run_bass_kernel_spmd  # noqa: E402

BF16 = mybir.dt.bfloat16
F32 = mybir.dt.float32
I16 = mybir.dt.int16
bfdt = ml_dtypes.bfloat16
AF = mybir.ActivationFunctionType
ALU = mybir.AluOpType

B, D, N = 4, 512, 2048
H, KVH, DH = 8, 2, 64
CONTEXT_LEN = 4096
NLOC = 1024  # tokens per core
P = 128
NCORES = 8
HP = H // 2  # head pairs
NCH = N // P  # 16 key chunks of 128
NCH_LOC = NLOC // P  # 8 local key chunks

# DVE/GPSIMD Schraudolph exp: bf16 bits = floor(s*SCH_A + SCH_B) (f32->i16
# convert truncates); C=6.0 calibrated for min max-rel-err (~3.7%).
SCH_A = 0.125 * 128.0 / float(np.log(2.0))
SCH_B = 16256.0 - 6.0

_CACHE = {}


def _enable_trace_hook():
    """Register the NTFF profile hook (missing antenv.axon_hooks shim)."""
    try:
        import antenv

        if "antenv.axon_hooks" in sys.modules:
            return
        mod = types.ModuleType("antenv.axon_hooks")

        def set_axon_ntff_profile_hook(h):
            mod._hook = h

        def get_axon_ntff_profile_hook():
            return getattr(mod, "_hook", None)

        mod.set_axon_ntff_profile_hook = set_axon_ntff_profile_hook
        mod.get_axon_ntff_profile_hook = get_axon_ntff_profile_hook
        sys.modules["antenv.axon_hooks"] = mod
        antenv.axon_hooks = mod
        from trn_agent_boot.trn_boot import _ntff_profile_via_ctypes

        set_axon_ntff_profile_hook(_ntff_profile_via_ctypes("/opt/axon/libaxon_pjrt.so"))
    except Exception:
        pass


def _swap_dma(nc, engs, dst, src):
    """dst[p] = src[p xor 32] via 4 SBUF->SBUF block DMAs."""
    for g in range(4):
        engs[g % len(engs)].dma_start(
            dst[32 * g : 32 * (g + 1), :], src[32 * (g ^ 1) : 32 * ((g ^ 1) + 1), :]
        )


def _build(TRIVIAL_GB, TRIVIAL_BO):
    nc = bacc.Bacc(None, target_bir_lowering=False, debug=False)
    dp = nc.declare_dram_parameter

    x_e = dp("x", [4, P, NLOC], BF16, isOutput=False)
    wq_e = dp("wq", [P, 4, 512], BF16, isOutput=False)
    wk_e = dp("wk", [P, 4, 128], BF16, isOutput=False)
    wv_e = dp("wv", [P, 4, 128], BF16, isOutput=False)
    wo_e = dp("wo", [P, 4, 512], BF16, isOutput=False)
    cq_e = dp("cq", [P, NLOC], BF16, isOutput=False)
    sq_e = dp("sq", [P, NLOC], BF16, isOutput=False)
    ck_e = dp("ck", [P, NLOC], BF16, isOutput=False)
    sk_e = dp("sk", [P, NLOC], BF16, isOutput=False)
    gam_e = dp("gam", [P, 4], F32, isOutput=False)  # gamma per (p, chunk)
    bet_e = dp("bet", [P, 4], F32, isOutput=False)  # beta per (p, chunk)
    bo_e = dp("bo", [P, 4], F32, isOutput=False)  # bout per (p, chunk)
    ones_e = dp("ones", [P, 1], F32, isOutput=False)
    out_e = dp("out", [4, P, NLOC], BF16, isOutput=True)

    with tile.TileContext(nc) as tc:
        with (
            tc.tile_pool(name="persist", bufs=1) as PS,
            tc.tile_pool(name="tmp", bufs=2) as TMP,
            tc.tile_pool(name="tmp4", bufs=4) as TMP4,
            tc.tile_pool(name="ep", bufs=4) as EP,
            tc.tile_pool(name="dram", bufs=1, space="DRAM") as DRAM,
        ):
            # ---------------- phase A: inputs -> SBUF ----------------
            SQP_cm = tc.tile_pool(name="sq_pool", bufs=1)
            SQP = SQP_cm.__enter__()
            # x as one [P, 4, 1024] bf16 tile; DMA split across issue queues
            onesw_sb = PS.tile([P, 128], BF16, name="onesw")
            nc.gpsimd.memset(onesw_sb[:], 1.0)
            x_sb = SQP.tile([P, 4, NLOC], BF16, name="xall")
            x_dmas = []
            iss = [nc.sync, nc.gpsimd]
            for c in range(4):
                for hq in range(4):
                    e = iss[(c * 4 + hq) % 2]  # sync/gpsimd rotate
                    x_dmas.append(
                        e.dma_start(
                            x_sb[:, c, ts(hq, 256)], x_e[c][:, ts(hq, 256)]
                        )
                    )
            ones_sb = PS.tile([P, 1], F32, name="ones")
            nc.sync.dma_start(ones_sb[:], ones_e[:])
            # preload the Ln table during input DMA (Exp set loads once at
            # the rstd step; attention exp then reuses it)
            warm = PS.tile([1, 1], F32, name="warm")
            nc.gpsimd.memset(warm[:], 1.0)
            nc.scalar.activation(warm[:], warm[:], AF.Ln)
            gam_sb = PS.tile([P, 4], F32, name="gam")
            nc.sync.dma_start(gam_sb[:], gam_e[:])
            bet_sb = PS.tile([P, 4], F32, name="bet")
            nc.sync.dma_start(bet_sb[:], bet_e[:])
            bo_sb = PS.tile([P, 4], F32, name="bo")
            nc.sync.dma_start(bo_sb[:], bo_e[:])
            wk_sb = PS.tile([P, 4, 128], BF16, name="wk")
            nc.sync.dma_start(wk_sb[:], wk_e[:])
            wv_sb = PS.tile([P, 4, 128], BF16, name="wv")
            nc.gpsimd.dma_start(wv_sb[:], wv_e[:])
            ck_sb = PS.tile([P, NLOC], BF16, name="ck")
            nc.sync.dma_start(ck_sb[:], ck_e[:])
            sk_sb = PS.tile([P, NLOC], BF16, name="sk")
            nc.sync.dma_start(sk_sb[:], sk_e[:])
            wq_sb = PS.tile([P, 4, 512], BF16, name="wq")
            for hq in range(2):
                nc.sync.dma_start(wq_sb[:, :, ts(hq, 256)], wq_e[:, :, ts(hq, 256)])
            cq_sb = PS.tile([P, NLOC], BF16, name="cq")
            nc.gpsimd.dma_start(cq_sb[:], cq_e[:])
            sq_sb = PS.tile([P, NLOC], BF16, name="sq")
            nc.gpsimd.dma_start(sq_sb[:], sq_e[:])
            wo_sb = PS.tile([P, 4, 512], BF16, name="wo")
            for hq in range(2):
                nc.gpsimd.dma_start(wo_sb[:, :, ts(hq, 256)], wo_e[:, :, ts(hq, 256)])

            # v lhsT stores: col DH is the ones column accumulating softmax den
            v_loc = PS.tile([P, 2 * NCH_LOC, DH + 1], BF16, name="vloc")
            nc.gpsimd.memset(v_loc[:, :, DH : DH + 1], 1.0)
            v_rem = [
                PS.tile([P, NCH_LOC, DH + 1], BF16, name=f"vrem{h}") for h in range(2)
            ]
            for h in range(2):
                nc.gpsimd.memset(v_rem[h][:, :, DH : DH + 1], 1.0)

            xnb = [PS.tile([P, NLOC], BF16, name=f"xnb{c}") for c in range(4)]
            qr_sb = [PS.tile([P, NLOC], BF16, name=f"qr{i}") for i in range(HP)]
            k_bf = PS.tile([P, NLOC], BF16, name="kbf")
            k_pre = PS.tile([P, NLOC], BF16, name="kpre")
            k_sw = PS.tile([P, NLOC], BF16, name="ksw")
            k_rem = [PS.tile([P, 512], BF16, name=f"krem{h}") for h in range(2)]
            vcp_sb = PS.tile([P, NCH_LOC, 128], BF16, name="vcp")
            ohat = [PS.tile([P, NLOC], BF16, name=f"oh{i}") for i in range(HP)]

            ag_in = DRAM.tile([2, P, NLOC], BF16)
            ag_out = DRAM.tile([2, 2, P, NLOC], BF16)

            # ---------------- phase B: layernorm ----------------
            with tc.tile_pool(name="ps_b1", bufs=1, space="PSUM") as PB1:
                warm_ps = PB1.tile([P, 512], F32, name="warmps")
                smu = PB1.tile([P, 2, 512], F32, name="smu")
                sx2 = PB1.tile([P, 2, 512], F32, name="sx2")
                for w8 in range(6):
                    nc.tensor.matmul(
                        warm_ps[:, 0:128], onesw_sb[:], onesw_sb[:],
                        start=True, stop=True,
                    )
                for w8 in range(4):
                    nc.tensor.matmul(
                        warm_ps[:, 0:256], onesw_sb[:], x_sb[:, 0, 0:256],
                        start=True, stop=True,
                    )
                xsq = SQP.tile([P, 4, NLOC], BF16, name="xsq")
                for c in range(4):
                    for h2 in range(2):
                        nc.vector.tensor_mul(
                            xsq[:, c, ts(h2, 512)],
                            x_sb[:, c, ts(h2, 512)],
                            x_sb[:, c, ts(h2, 512)],
                        )
                for tq in range(2):
                    for c in range(4):
                        nc.tensor.matmul(
                            smu[:, tq, :], onesw_sb[:], x_sb[:, c, ts(tq, 512)],
                            start=(c == 0), stop=(c == 3),
                        )
                for tq in range(2):
                    for c in range(4):
                        nc.tensor.matmul(
                            sx2[:, tq, :], onesw_sb[:], xsq[:, c, ts(tq, 512)],
                            start=(c == 0), stop=(c == 3),
                        )
                mu_w = TMP.tile([P, NLOC], F32, tag="lnw", bufs=1)
                musq = TMP.tile([P, NLOC], F32, tag="lnw2", bufs=1)
                var_w = TMP.tile([P, NLOC], F32, tag="lnw3", bufs=1)
                sd_w = TMP.tile([P, NLOC], F32, tag="lnw4", bufs=1)
                rstd_bc = SQP.tile([P, NLOC], BF16, name="rstdbc")
                mrs_bc = SQP.tile([P, NLOC], BF16, name="mrsbc")
                # Ln for both halves first, then Exp (1 ACT table switch)
                for h2 in range(2):
                    sl = ts(h2, 512)
                    nc.vector.tensor_scalar_mul(mu_w[:, sl], smu[:, h2, :], 1.0 / 512.0)
                    nc.vector.tensor_mul(musq[:, sl], mu_w[:, sl], mu_w[:, sl])
                    nc.vector.scalar_tensor_tensor(
                        var_w[:, sl], sx2[:, h2, :], 1.0 / 512.0,
                        musq[:, sl], ALU.mult, ALU.subtract,
                    )
                    nc.scalar.activation(sd_w[:, sl], var_w[:, sl], AF.Ln)
                for h2 in range(2):
                    sl = ts(h2, 512)
                    nc.scalar.activation(rstd_bc[:, sl], sd_w[:, sl], AF.Exp, scale=-0.5)
                    nc.vector.scalar_tensor_tensor(
                        mrs_bc[:, sl], smu[:, h2, :], 1.0 / 512.0,
                        rstd_bc[:, sl], ALU.mult, ALU.mult,
                    )

                # xn per token-half; k projection starts as soon as its half
                # of xn is ready (fills the PE during the LN elementwise ops)
                kp = PB1.tile([P, 2, 512], F32, name="kp")
                for tq in range(2):
                    for c in range(4):
                        t1 = TMP.tile([P, 512], BF16, tag="th")
                        nc.vector.tensor_mul(
                            t1[:], x_sb[:, c, ts(tq, 512)], rstd_bc[:, ts(tq, 512)]
                        )
                        if TRIVIAL_GB:
                            nc.vector.tensor_tensor(
                                xnb[c][:, ts(tq, 512)], t1[:],
                                mrs_bc[:, ts(tq, 512)], ALU.subtract,
                            )
                        else:
                            t2 = TMP.tile([P, 512], BF16, tag="th")
                            nc.vector.tensor_tensor(
                                t2[:], t1[:], mrs_bc[:, ts(tq, 512)], ALU.subtract
                            )
                            nc.vector.tensor_scalar(
                                xnb[c][:, ts(tq, 512)], t2[:],
                                gam_sb[:, c : c + 1], bet_sb[:, c : c + 1],
                                ALU.mult, ALU.add,
                            )
                    for c in range(4):
                        nc.tensor.matmul(
                            kp[:, tq, :], wk_sb[:, c, :], xnb[c][:, ts(tq, 512)],
                            start=(c == 0), stop=(c == 3),
                        )
                    nc.vector.tensor_copy(k_pre[:, ts(tq, 512)], kp[:, tq, :])
            SQP_cm.__exit__(None, None, None)

            # ---------------- phase C1: k rotary, v projection, allgather ----------------
            with tc.tile_pool(name="ps_c1", bufs=1, space="PSUM") as PC:
                _swap_dma(nc, (nc.gpsimd, nc.sync), k_sw, k_pre)
                t1 = TMP.tile([P, NLOC], BF16, tag="t")
                t2 = TMP.tile([P, NLOC], BF16, tag="t")
                nc.vector.tensor_mul(t1[:], ck_sb[:], k_pre[:])
                nc.vector.tensor_mul(t2[:], sk_sb[:], k_sw[:])
                nc.vector.tensor_add(k_bf[:], t1[:], t2[:])

                for c8 in range(NCH_LOC):
                    vp = PC.tile([P, 128], F32, name=f"vp{c8 % 2}")
                    for c in range(4):
                        nc.tensor.matmul(
                            vp[:], xnb[c][:, ts(c8, 128)], wv_sb[:, c, :],
                            start=(c == 0), stop=(c == 3),
                        )
                    nc.vector.tensor_copy(vcp_sb[:, c8, :], vp[:])

                nc.sync.dma_start(ag_in[0], k_bf[:])
                nc.sync.dma_start(
                    ag_in[1], vcp_sb[:].rearrange("p a b -> p (a b)")
                )
                nc.gpsimd.collective_compute(
                    "AllGather",
                    ALU.bypass,
                    ins=[ag_in[:]],
                    outs=[ag_out[:]],
                    replica_groups=[[0, 1], [2, 3], [4, 5], [6, 7]],
                )

            # local v -> bf16 slots 0..15 (gpsimd; off critical path)
            nc.gpsimd.tensor_copy(
                v_loc[:, :, 0:DH],
                vcp_sb[:].rearrange("p a (g d) -> p (a g) d", g=2),
            )

            # ---------------- phase D: attention main loop ----------------
            # Groups of 2 slots (PSUM: sc [P,2,512] x2 bufs + AV pairs x2 bufs
            # = 8 banks). Exp engine round-robins ACT/ACT/DVE per group.
            spills = {}
            exp_rr = {"i": 0}
            EXP_PAT = ("act", "act", "act", "dve", "act", "act", "act", "act", "dve")

            def emit_qproj(PSC, i, copy_eng):
                qps = PSC.tile([P, 2, 512], F32, tag="sc", name="qps")
                for tq in range(2):
                    for c in range(4):
                        nc.tensor.matmul(
                            qps[:, tq, :], wq_sb[:, c, ts(i, 128)],
                            xnb[c][:, ts(tq, 512)],
                            start=(c == 0), stop=(c == 3),
                        )
                qc = TMP4.tile([P, NLOC], BF16, tag="qcs")
                qs = TMP4.tile([P, NLOC], BF16, tag="qcs")
                for tq in range(2):
                    copy_eng(qc[:, ts(tq, 512)], qps[:, tq, :])
                _swap_dma(nc, (nc.sync, nc.gpsimd), qs, qc)
                t1 = TMP.tile([P, NLOC], BF16, tag="qt")
                t2 = TMP.tile([P, NLOC], BF16, tag="qt")
                if i < 2:
                    nc.vector.tensor_mul(t1[:], cq_sb[:], qc[:])
                    nc.vector.tensor_mul(t2[:], sq_sb[:], qs[:])
                    nc.vector.tensor_add(qr_sb[i][:], t1[:], t2[:])
                else:
                    nc.gpsimd.tensor_mul(t1[:], cq_sb[:], qc[:])
                    nc.gpsimd.tensor_mul(t2[:], sq_sb[:], qs[:])
                    nc.gpsimd.tensor_add(qr_sb[i][:], t1[:], t2[:])

            def emit_epilogue(hp, tq, oA, oB, restore, mul_gps=False):
                sAB = TMP.tile([DH + 1, 2, 512], BF16, tag="sum")
                if restore:
                    cpA, cpB = spills[(hp, tq)]
                    nc.vector.tensor_add(sAB[:, 0, :], oA[:], cpA[:])
                    nc.vector.tensor_add(sAB[:, 1, :], oB[:], cpB[:])
                else:
                    nc.vector.tensor_copy(sAB[:, 0, :], oA[:])
                    nc.vector.tensor_copy(sAB[:, 1, :], oB[:])
                den2 = TMP.tile([1, 1024], F32, tag="den", bufs=2)
                nc.vector.tensor_copy(
                    den2[0:1, :], sAB[DH : DH + 1, :, :].rearrange("p a b -> p (a b)")
                )
                db = TMP.tile([64, 1024], F32, tag="db", bufs=2)
                nc.gpsimd.partition_broadcast(db[:], den2[0:1, :])
                pb = TMP.tile([64, 1024], F32, tag="pb", bufs=2)
                nc.vector.reciprocal_approx_fast(pb[:], db[:])
                nc.vector.tensor_mul(ohat[hp][0:64, ts(tq, 512)], sAB[0:DH, 0, :], pb[:, 0:512])
                nc.vector.tensor_mul(ohat[hp][64:128, ts(tq, 512)], sAB[0:DH, 1, :], pb[:, 512:1024])

            def emit_spill(hp, tq, oA, oB):
                cpA = PS.tile([DH + 1, 512], BF16, name=f"spA{hp}{tq}")
                cpB = PS.tile([DH + 1, 512], BF16, name=f"spB{hp}{tq}")
                nc.vector.tensor_copy(cpA[:], oA[:])
                nc.vector.tensor_copy(cpB[:], oB[:])
                spills[(hp, tq)] = (cpA, cpB)

            def run_stream(PSC, PAV, plan, hook=None):
                """plan: list of (hp, tq, chunks, mode).

                Groups of 2 slots; exp engine per group follows EXP_PAT
                round-robin. Pipeline: scores | AV of ready groups | exp.
                """
                allslots = []
                for hp, tq, chunks, mode in plan:
                    nunits = 2 * len(chunks)
                    seg = {
                        "hp": hp, "tq": tq, "mode": mode,
                        "nunits": nunits, "done_units": {0: 0, 1: 0},
                        "oA": None, "oB": None, "mul_gps": mode == "epi",
                    }
                    for ci, ch in enumerate(chunks):
                        for par in range(2):
                            allslots.append((seg, par, ci, ch))
                groups = [allslots[i : i + 2] for i in range(0, len(allslots), 2)]

                def emit_av_unit(seg, par, ch, rhs, first, last):
                    if first:
                        if par == 0:
                            seg["oA"] = PAV.tile([DH + 1, 512], F32, tag="avA", name="av_a")
                        else:
                            seg["oB"] = PAV.tile([DH + 1, 512], F32, tag="avB", name="av_b")
                    o = seg["oA"] if par == 0 else seg["oB"]
                    if ch < NCH_LOC:
                        vt = v_loc[:, 2 * ch + par, :]
                    else:
                        cr = ch - NCH_LOC
                        vt = v_rem[cr // 4][:, 2 * (cr % 4) + par, :]
                    nc.tensor.matmul(o[:], vt, rhs, start=first, stop=last)
                    seg["done_units"][par] += 1
                    if (
                        seg["done_units"][0] + seg["done_units"][1]
                        == seg["nunits"]
                    ):
                        oA, oB = seg["oA"], seg["oB"]
                        if seg["mode"] == "spill":
                            emit_spill(seg["hp"], seg["tq"], oA, oB)
                        else:
                            emit_epilogue(
                                seg["hp"], seg["tq"], oA, oB,
                                seg["mode"] == "epi_restore",
                                mul_gps=seg["mul_gps"],
                            )

                pending = []  # (trigger_gidx, emit_fn)
                hook_at = len(groups) // 2
                for gidx, slots in enumerate(groups + [None] * 2):
                    if gidx == hook_at and hook is not None:
                        hook()
                    if slots is not None:
                        sc = PSC.tile([P, 2, 512], F32, tag="sc")
                        for pos, (sseg, par, ci, ch) in enumerate(slots):
                            hp, tq = sseg["hp"], sseg["tq"]
                            if ch < NCH_LOC:
                                ksrc = k_bf[:, ts(ch, 128)]
                            else:
                                cr = ch - NCH_LOC
                                ksrc = k_rem[cr // 4][:, ts(cr % 4, 128)]
                            nc.tensor.matmul(
                                sc[:, pos, :],
                                ksrc[64 * par : 64 * (par + 1), :],
                                qr_sb[hp][64 * par : 64 * (par + 1), ts(tq, 512)],
                                start=True, stop=True,
                                tile_position=(64 * par, 0),
                            )
                    while pending and pending[0][0] < gidx:
                        pending.pop(0)[1]()
                    if slots is not None:
                        ns = len(slots)
                        eng = EXP_PAT[exp_rr["i"] % len(EXP_PAT)]
                        exp_rr["i"] += 1
                        et = EP.tile([P, 2, 512], BF16, tag="ep", name="etg")
                        if eng == "act":
                            nc.scalar.activation(
                                et[:, 0:ns, :].rearrange("p a b -> p (a b)"),
                                sc[:, 0:ns, :].rearrange("p a b -> p (a b)"),
                                AF.Exp, scale=0.125,
                            )
                        else:
                            nc.vector.tensor_scalar(
                                et[:, 0:ns, :]
                                .rearrange("p a b -> p (a b)")
                                .bitcast(I16),
                                sc[:, 0:ns, :].rearrange("p a b -> p (a b)"),
                                SCH_A, SCH_B, ALU.mult, ALU.add,
                            )
                        for pos, (seg_, par, ci, ch) in enumerate(slots):
                            rhs = et[:, pos, :]
                            nu = seg_["sched_units"] = seg_.get("sched_units", {0: 0, 1: 0})
                            nu[par] += 1
                            first_u = nu[par] == 1
                            last_u = nu[par] == seg_["nunits"] // 2
                            pending.append(
                                (
                                    gidx,
                                    (lambda s=seg_, p=par, c=ch, r=rhs,
                                     f=first_u, l=last_u: emit_av_unit(
                                        s, p, c, r, f, l
                                    )),
                                )
                            )
                while pending:
                    pending.pop(0)[1]()

            LOC = list(range(NCH_LOC))
            REM = list(range(NCH_LOC, NCH))
            with (
                tc.tile_pool(name="ps_sc", bufs=2, space="PSUM") as PSC,
                tc.tile_pool(name="ps_av", bufs=2, space="PSUM") as PAV,
            ):
                emit_qproj(PSC, 0, nc.vector.tensor_copy)
                emit_qproj(PSC, 1, nc.vector.tensor_copy)
                run_stream(PSC, PAV, [(0, 0, LOC, "spill")],
                           hook=lambda: emit_qproj(PSC, 2, nc.vector.tensor_copy))
                run_stream(PSC, PAV, [(0, 1, LOC, "spill")],
                           hook=lambda: emit_qproj(PSC, 3, nc.vector.tensor_copy))
                run_stream(PSC, PAV, [(1, 0, LOC, "spill")])
                run_stream(PSC, PAV, [(1, 1, LOC, "spill")])

                # remote kv recovery: remote = (ag0 + ag1) - local  (exact)
                for h in range(2):
                    agk0 = TMP.tile([P, 512], BF16, tag="ag")
                    agk1 = TMP.tile([P, 512], BF16, tag="ag")
                    nc.sync.dma_start(agk0[:], ag_out[0, 0][:, ts(h, 512)])
                    nc.gpsimd.dma_start(agk1[:], ag_out[1, 0][:, ts(h, 512)])
                    tk = TMP.tile([P, 512], F32, tag="th")
                    nc.vector.tensor_add(tk[:], agk0[:], agk1[:])
                    nc.vector.tensor_tensor(
                        k_rem[h][:], tk[:], k_bf[:, ts(h, 512)], ALU.subtract
                    )
                for h in range(2):
                    agv0 = TMP.tile([P, 512], BF16, tag="ag")
                    agv1 = TMP.tile([P, 512], BF16, tag="ag")
                    nc.sync.dma_start(agv0[:], ag_out[0, 1][:, ts(h, 512)])
                    nc.gpsimd.dma_start(agv1[:], ag_out[1, 1][:, ts(h, 512)])
                    tv = TMP.tile([P, 512], F32, tag="th")
                    if h == 0:
                        nc.vector.tensor_add(tv[:], agv0[:], agv1[:])
                        nc.vector.tensor_tensor(
                            v_rem[h][:, :, 0:DH],
                            tv[:].rearrange("p (a g d) -> p (a g) d", g=2, d=DH),
                            vcp_sb[:, ts(h, 4), :].rearrange("p a (g d) -> p (a g) d", g=2),
                            ALU.subtract,
                        )
                    else:
                        nc.vector.tensor_add(tv[:], agv0[:], agv1[:])
                        nc.vector.tensor_tensor(
                            v_rem[h][:, :, 0:DH],
                            tv[:].rearrange("p (a g d) -> p (a g) d", g=2, d=DH),
                            vcp_sb[:, ts(h, 4), :].rearrange("p a (g d) -> p (a g) d", g=2),
                            ALU.subtract,
                        )

                run_stream(PSC, PAV, [
                    (2, 0, LOC + REM, "epi"),
                    (2, 1, LOC + REM, "epi"),
                ])
                run_stream(PSC, PAV, [
                    (3, 0, LOC + REM, "epi"),
                    (3, 1, LOC + REM, "epi"),
                ])
                run_stream(PSC, PAV, [
                    (1, 1, REM, "epi_restore"),
                    (0, 1, REM, "epi_restore"),
                ])
                run_stream(PSC, PAV, [
                    (1, 0, REM, "epi_restore"),
                    (0, 0, REM, "epi_restore"),
                ])

            # ---------------- phase E: output projection + residual ----------------
            # tq=1 first: its ohat tiles complete one stream earlier
            with tc.tile_pool(name="ps_e", bufs=8, space="PSUM") as PE_:
                def emit_y(tq, mc, yps):
                    yt = TMP.tile([P, 512], BF16, tag="yout")
                    nc.vector.tensor_add(yt[:], yps[:], xnb[mc][:, ts(tq, 512)])
                    if TRIVIAL_BO:
                        yo = yt
                    else:
                        yo = TMP.tile([P, 512], BF16, tag="yout")
                        nc.vector.tensor_scalar_add(
                            yo[:], yt[:], bo_sb[:, mc : mc + 1]
                        )
                    for dq in range(2):
                        (nc.sync, nc.gpsimd)[dq].dma_start(
                            out_e[mc, :, ds(tq * 512 + dq * 256, 256)],
                            yo[:, ts(dq, 256)],
                        )

                # tq=1: ohat ready after stream 9 -> single pass
                for mc in range(4):
                    yps = PE_.tile([P, 512], F32, tag="yps")
                    for kc in (3, 2, 1, 0):
                        nc.tensor.matmul(
                            yps[:], wo_sb[:, kc, ts(mc, 128)],
                            ohat[kc][:, ts(1, 512)],
                            start=(kc == 3), stop=(kc == 0),
                        )
                    emit_y(1, mc, yps)
                # tq=0: kc 3,2,1 first for all mc (ready early), kc 0 last
                yps0 = []
                for mc in range(4):
                    yps = PE_.tile([P, 512], F32, tag="yps")
                    yps0.append(yps)
                    for kc in (3, 2, 1):
                        nc.tensor.matmul(
                            yps[:], wo_sb[:, kc, ts(mc, 128)],
                            ohat[kc][:, ts(0, 512)],
                            start=(kc == 3), stop=False,
                        )
                for mc in range(4):
                    nc.tensor.matmul(
                        yps0[mc][:], wo_sb[:, 0, ts(mc, 128)],
                        ohat[0][:, ts(0, 512)],
                        start=False, stop=True,
                    )
                for mc in range(4):
                    emit_y(0, mc, yps0[mc])

    nc.compile()
    return nc


def _host_inputs(x, gamma, beta, Wq, Wkv, Wout, bout):
    """Build the 8 per-core input maps."""
    x = np.asarray(x, np.float32)
    gamma = np.asarray(gamma, np.float32)
    beta = np.asarray(beta, np.float32)
    Wq = np.asarray(Wq, np.float32)
    Wkv = np.asarray(Wkv, np.float32)
    Wout = np.asarray(Wout, np.float32)
    bout = np.asarray(bout, np.float32)

    def lhsT(W):
        # [D, M] -> [128, 4, M] chunk layout
        return np.ascontiguousarray(
            W.reshape(4, P, W.shape[1]).transpose(1, 0, 2).astype(bfdt)
        )

    Wk = Wkv[:, : KVH * DH]
    Wv = Wkv[:, KVH * DH :]
    wq = lhsT(Wq)
    wk = lhsT(Wk)
    wv = lhsT(Wv)
    wo = lhsT(Wout)
    gam = np.ascontiguousarray(gamma.reshape(4, P).T)
    bet = np.ascontiguousarray(beta.reshape(4, P).T)
    bo = np.ascontiguousarray(bout.reshape(4, P).T)
    ones = np.ones((P, 1), np.float32)

    # rotary tables (per half)
    j = np.arange(DH)
    inv_freq = 1.0 / (10000.0 ** ((2.0 * (j % 32)) / DH))
    base = ((2.0 * (j % 32)) + 0.4 * DH) / (1.4 * DH)
    sign = np.where(j < 32, -1.0, 1.0)

    tables = []
    for half in range(2):
        pos = half * NLOC + np.arange(NLOC, dtype=np.float64)
        freqs = pos[None, :] * inv_freq[:, None]  # [64, NLOC]
        cos, sin = np.cos(freqs), np.sin(freqs)
        power = (pos - N // 2) / CONTEXT_LEN
        xsc = base[:, None] ** power[None, :]
        cq = np.tile((cos * xsc), (2, 1)).astype(bfdt)
        sq = np.tile((sign[:, None] * sin * xsc), (2, 1)).astype(bfdt)
        ck = np.tile((cos / xsc), (2, 1)).astype(bfdt)
        sk = np.tile((sign[:, None] * sin / xsc), (2, 1)).astype(bfdt)
        tables.append((cq, sq, ck, sk))

    in_maps = []
    for core in range(NCORES):
        b, half = core // 2, core % 2
        xc = np.ascontiguousarray(
            x[b].reshape(4, P, N)[:, :, half * NLOC : (half + 1) * NLOC]
        ).astype(bfdt)
        cq, sq, ck, sk = tables[half]
        in_maps.append(
            {
                "x": xc, "wq": wq, "wk": wk,
                "wv": wv, "wo": wo, "cq": cq, "sq": sq, "ck": ck, "sk": sk,
                "gam": gam, "bet": bet, "bo": bo, "ones": ones,
            }
        )
    return in_maps


def kernel(x, gamma, beta, Wq, Wkv, Wout, bout):
    trace = os.environ.get("KERNEL_TRACE", "0") == "1"
    if trace:
        _enable_trace_hook()
    trivial_gb = bool(
        np.all(np.asarray(gamma) == 1.0) and np.all(np.asarray(beta) == 0.0)
    )
    trivial_bo = bool(np.all(np.asarray(bout) == 0.0))
    if "nc" not in _CACHE:
        _CACHE["nc"] = _build(trivial_gb, trivial_bo)
        _CACHE["trivial_gb"] = (trivial_gb, trivial_bo)
    assert _CACHE["trivial_gb"] == (trivial_gb, trivial_bo)
    nc = _CACHE["nc"]
    in_maps = _host_inputs(x, gamma, beta, Wq, Wkv, Wout, bout)
    res = run_bass_kernel_spmd(nc, in_maps, list(range(NCORES)), trace=trace)
    if trace and res.exec_time_ns is not None:
        print(f"HW exec time: {res.exec_time_ns} ns")
        _CACHE["exec_time_ns"] = res.exec_time_ns

    y = np.empty((B, D, N), np.float32)
    for core in range(NCORES):
        b, half = core // 2, core % 2
        y[b, :, half * NLOC : (half + 1) * NLOC] = (
            np.asarray(res.results[core]["out"]).astype(np.float32).reshape(D, NLOC)
        )
    return y
